# revision 2
# baseline (speedup 1.0000x reference)
"""Bipartite GNN (factor -> variable) message passing on 8 Trainium2 NeuronCores.

V7: edge-major dataflow with big multi-packet SWDGE gathers.

Per core (disjoint 12500-variable slices, no collectives):
  prologue:  yv_stage[v,:] = V_blk @ Wm_top           (bf16 row-major DRAM)
             zf_stage[r,:] = F @ Wm_bot + bm          (bf16 row-major DRAM)
  edge phase (chunks of 128 edges, senders in one 128-var block, blocks
  contiguous; batches of GB chunks):
    yb = dma_gather(yv_stage, senders)   SWDGE, 1 call/batch, multi-packet
    zb = dma_gather(zf_stage, r)         SWDGE, 1 call/batch
    m   = yb + zb                        DVE   (wide)
    msg = relu(m)                        ACT   (wide)
    aggT_k += msg_c^T @ gs_c             TensorE, one closed PSUM group per
                                         block; gs streamed from host in fp8
  combine (transposed frame, 4 blocks / 512 vars at a time):
    hT = relu(Wc_top^T @ V^T + Wc_bot^T @ aggT + bc); outT = vT + hT
  output written feature-major; host transposes.
"""

import numpy as np
import ml_dtypes

BF16 = ml_dtypes.bfloat16
FP8 = ml_dtypes.float8_e4m3fn

N_VAR, N_FAC, N_EDGE = 100000, 50000, 1000000
N_CORES = 8
SLOT_INVALID = 255

GB = 64    # chunks per gather batch (8192 edges)
ZBASE = 32768  # int16 base shift for zf gather indices
WIN = 4    # chunks per formation/elementwise window
GS_FP8 = True


def _cdiv(a, b):
    return -(-a // b)


def _make_plan(senders, receivers):
    send = np.asarray(senders).astype(np.int64).ravel()
    recv = np.asarray(receivers).astype(np.int64).ravel()
    vpc = N_VAR // N_CORES
    nblk = _cdiv(vpc, 128)

    counts = np.zeros((N_CORES, nblk), np.int64)
    per_core = []
    for c in range(N_CORES):
        lo = c * vpc
        m = (send >= lo) & (send < lo + vpc)
        s = (send[m] - lo).astype(np.int64)
        r = recv[m].astype(np.int64)
        blk = s >> 7
        o = np.argsort(blk, kind="stable")
        s, r, blk = s[o], r[o], blk[o]
        counts[c] = np.bincount(blk, minlength=nblk)
        per_core.append((s, r, blk))

    q = np.maximum(1, _cdiv(counts, 128).max(axis=0))  # [nblk]
    blk_start = np.zeros(nblk + 1, np.int64)
    blk_start[1:] = np.cumsum(q)
    Q = int(blk_start[-1])

    core_data = []
    for c in range(N_CORES):
        s, r, blk = per_core[c]
        slot_arr = np.full(Q * 128, SLOT_INVALID, np.int64)
        yidx_arr = np.zeros(Q * 128, np.int64)
        zidx_arr = np.zeros(Q * 128, np.int64)
        first_of_blk = np.zeros(nblk, np.int64)
        cnt = np.bincount(blk, minlength=nblk)
        first_of_blk[1:] = np.cumsum(cnt)[:-1]
        pos = blk_start[blk] * 128 + (np.arange(s.shape[0]) - first_of_blk[blk])
        slot_arr[pos] = s & 127
        yidx_arr[pos] = s
        zidx_arr[pos] = r - ZBASE

        # trailing negative zf indices terminate a gather call early: the
        # last stream position of every GB-chunk call must be non-negative.
        for b0 in range(0, Q, GB):
            bn = min(GB, Q - b0)
            last = (b0 + bn) * 128 - 1
            if zidx_arr[last] >= 0:
                continue
            lo = last - 127
            cand = np.where(zidx_arr[lo : last + 1] >= 0)[0]
            assert cand.size > 0, "gather tail chunk has no non-negative zf idx"
            j = lo + cand[-1]
            for arr in (slot_arr, yidx_arr, zidx_arr):
                arr[last], arr[j] = arr[j], arr[last]

        def wrap16(a):
            w = a.reshape(Q * 8, 16).T.astype(np.int16)
            return np.tile(w, (8, 1))

        gs_dt = FP8 if GS_FP8 else BF16
        gs = np.zeros((128, Q * 128), dtype=gs_dt)
        valid = np.where(slot_arr != SLOT_INVALID)[0]
        vslot = slot_arr[valid]
        gs[valid % 128, (valid & ~np.int64(127)) + vslot] = 1
        gsT = np.zeros((128, Q * 128), dtype=gs_dt)
        gsT[vslot, valid] = 1

        core_data.append(dict(zidx=wrap16(zidx_arr), gs=gs, gsT=gsT))

    st = dict(
        vpc=vpc,
        nblk=nblk,
        vpad=nblk * 128,
        fpad=_cdiv(N_FAC, 128) * 128,
        Q=Q,
        q=[int(x) for x in q],
        blk_start=[int(x) for x in blk_start],
    )
    return st, core_data


def _build_program(st):
    import concourse.mybir as mybir
    from concourse import bacc
    from concourse.tile import TileContext

    dt = mybir.dt
    f32, bf16, i16 = dt.float32, dt.bfloat16, dt.int16
    gsdt = dt.float8e4 if GS_FP8 else bf16
    AF = mybir.ActivationFunctionType
    ALU = mybir.AluOpType

    vpc, nblk, vpad, fpad, Q = (
        st["vpc"], st["nblk"], st["vpad"], st["fpad"], st["Q"],
    )
    q, blk_start = st["q"], st["blk_start"]

    nc = bacc.Bacc(None, target_bir_lowering=False)

    p_vt = nc.declare_dram_parameter("vt", [128, vpad], bf16, isOutput=False)
    p_ft = nc.declare_dram_parameter("ft", [128, fpad], bf16, isOutput=False)
    p_wm_top = nc.declare_dram_parameter("wm_top", [128, 128], bf16, isOutput=False)
    p_wm_bot = nc.declare_dram_parameter("wm_bot", [128, 128], bf16, isOutput=False)
    p_wc_top = nc.declare_dram_parameter("wc_top", [128, 128], bf16, isOutput=False)
    p_wc_bot = nc.declare_dram_parameter("wc_bot", [128, 128], bf16, isOutput=False)
    p_bm4 = nc.declare_dram_parameter("bm4", [128, 512], bf16, isOutput=False)
    p_bc = nc.declare_dram_parameter("bc_col", [128, 1], f32, isOutput=False)
    p_zidx = nc.declare_dram_parameter("zidx", [128, Q * 8], i16, isOutput=False)
    p_gs = nc.declare_dram_parameter("gs", [128, Q * 128], gsdt, isOutput=False)
    p_gsT = nc.declare_dram_parameter("gsT", [128, Q * 128], gsdt, isOutput=False)
    p_out = nc.declare_dram_parameter("out_t", [128, vpad], f32, isOutput=True)

    zf_stage = nc.dram_tensor("zf_stage", [fpad, 128], bf16)

    # batches of GB chunks
    batches = [(b0, min(GB, Q - b0)) for b0 in range(0, Q, GB)]

    with TileContext(nc) as tc:
        with (
            tc.tile_pool(name="const", bufs=1) as cpool,
            tc.tile_pool(name="vts", bufs=2) as vtspool,
            tc.tile_pool(name="fts", bufs=2) as ftspool,
            tc.tile_pool(name="psA", bufs=3, space="PSUM") as psA,
            tc.tile_pool(name="psC", bufs=3, space="PSUM") as psC,
            tc.tile_pool(name="psD", bufs=2, space="PSUM") as psD,
            tc.tile_pool(name="stage", bufs=3) as stpool,
            tc.tile_pool(name="strm", bufs=4) as spool,
            tc.tile_pool(name="gath", bufs=4) as gpool,
            tc.tile_pool(name="work", bufs=4) as wpool,
            tc.tile_pool(name="aggb", bufs=2) as aggbpool,
            tc.tile_pool(name="outb", bufs=2) as opool,
        ):
            def load_const(name, param, shape, dtype):
                t = cpool.tile(shape, dtype, tag=name)
                nc.sync.dma_start(out=t[:], in_=param[:, :])
                return t

            wm_top_sb = load_const("wm_top", p_wm_top, [128, 128], bf16)
            wm_bot_sb = load_const("wm_bot", p_wm_bot, [128, 128], bf16)
            wc_top_sb = load_const("wc_top", p_wc_top, [128, 128], bf16)
            wc_bot_sb = load_const("wc_bot", p_wc_bot, [128, 128], bf16)
            bc_sb = load_const("bc_col", p_bc, [128, 1], f32)
            bm4_sb = load_const("bm4", p_bm4, [128, 512], bf16)

            # ---- prologue: zf_stage = F @ Wm_bot + bm (row-major DRAM) ----
            for J in range(0, fpad, 2048):
                wj = min(2048, fpad - J)
                ftt = ftspool.tile([128, 2048], bf16, tag="ftt")
                nc.sync.dma_start(out=ftt[:, :wj], in_=p_ft[:, J : J + wj])
                for j in range(0, wj, 512):
                    w = min(512, wj - j)
                    ps = psA.tile([128, 512], f32, tag="psA")
                    for kk in range(0, w, 128):
                        nc.tensor.matmul(
                            out=ps[:, kk : kk + 128],
                            lhsT=ftt[:, j + kk : j + kk + 128],
                            rhs=wm_bot_sb[:],
                            start=True,
                            stop=True,
                        )
                    stg = stpool.tile([128, 512], bf16, tag="stg")
                    nc.vector.tensor_tensor(
                        out=stg[:, :w], in0=ps[:, :w], in1=bm4_sb[:, :w], op=ALU.add
                    )
                    nc.sync.dma_start(
                        out=zf_stage[J + j : J + j + w, :].rearrange(
                            "(a p) b -> p a b", p=128
                        ),
                        in_=stg[:, :w].rearrange("p (a b) -> p a b", b=128),
                    )

            yvB = cpool.tile([128, vpad], bf16, tag="yvB")
            # ---- prologue: yvB blocks = V_blk @ Wm_top (SBUF) ----
            for j in range(0, vpad, 512):
                w = min(512, vpad - j)
                vts = vtspool.tile([128, 512], bf16, tag="vts")
                nc.sync.dma_start(out=vts[:, :w], in_=p_vt[:, j : j + w])
                ps = psA.tile([128, 512], f32, tag="psA")
                for kk in range(0, w, 128):
                    nc.tensor.matmul(
                        out=ps[:, kk : kk + 128],
                        lhsT=vts[:, kk : kk + 128],
                        rhs=wm_top_sb[:],
                        start=True,
                        stop=True,
                    )
                nc.scalar.copy(out=yvB[:, j : j + w], in_=ps[:, :w])

            # ---- edge phase ----
            heads = {}

            def emit_head(bi):
                b0, bn = batches[bi]
                zi = spool.tile([128, GB * 8], i16, tag="zi")
                nc.sync.dma_start(
                    out=zi[:, : bn * 8], in_=p_zidx[:, b0 * 8 : (b0 + bn) * 8]
                )
                gst = spool.tile([128, GB * 128], gsdt, tag="gst")
                nc.sync.dma_start(
                    out=gst[:, : bn * 128],
                    in_=p_gs[:, b0 * 128 : (b0 + bn) * 128],
                )
                gsTt = spool.tile([128, GB * 128], gsdt, tag="gsTt")
                nc.sync.dma_start(
                    out=gsTt[:, : bn * 128],
                    in_=p_gsT[:, b0 * 128 : (b0 + bn) * 128],
                )
                zb = gpool.tile([128, GB, 128], bf16, tag="zb")
                nc.gpsimd.dma_gather(
                    out_ap=zb[:, :bn, :],
                    in_ap=zf_stage[ZBASE:, :],
                    idxs_ap=zi[:, : bn * 8],
                    num_idxs=bn * 128,
                    num_idxs_reg=bn * 128,
                    elem_size=128,
                    single_packet=False,
                )
                heads[bi] = (gst, gsTt, zb)

            # chunk -> block map
            blk_of_chunk = np.zeros(Q, np.int64)
            for k in range(nblk):
                blk_of_chunk[blk_start[k] : blk_start[k + 1]] = k

            # block close / combine machinery
            aggb_of_grp = {}
            grp_closed = {}

            def block_done(k, agg_ps):
                grp = k // 4
                if grp not in aggb_of_grp:
                    aggb_new = aggbpool.tile([128, 512], bf16, tag="aggb")
                    aggb_of_grp[grp] = aggb_new
                aggb = aggb_of_grp[grp]
                sl = slice((k % 4) * 128, (k % 4 + 1) * 128)
                nc.scalar.copy(out=aggb[:, sl], in_=agg_ps[:])
                grp_closed[grp] = grp_closed.get(grp, 0) + 1
                size = min(4, nblk - grp * 4)
                if grp_closed[grp] != size:
                    return
                aggb = aggb_of_grp.pop(grp)
                j0 = grp * 512
                w = size * 128
                vtc = vtspool.tile([128, 512], bf16, tag="vts")
                nc.sync.dma_start(out=vtc[:, :w], in_=p_vt[:, j0 : j0 + w])
                hps = psD.tile([128, 512], f32, tag="hps")
                nc.tensor.matmul(
                    out=hps[:, :w], lhsT=wc_top_sb[:], rhs=vtc[:, :w],
                    start=True, stop=False,
                )
                nc.tensor.matmul(
                    out=hps[:, :w], lhsT=wc_bot_sb[:], rhs=aggb[:, :w],
                    start=False, stop=True,
                )
                ht = opool.tile([128, 512], bf16, tag="ht")
                nc.scalar.activation(
                    out=ht[:, :w], in_=hps[:, :w], func=AF.Relu, bias=bc_sb[:]
                )
                ot = opool.tile([128, 512], f32, tag="ot")
                nc.vector.tensor_tensor(
                    out=ot[:, :w], in0=ht[:, :w], in1=vtc[:, :w], op=ALU.add
                )
                nc.sync.dma_start(out=p_out[:, j0 : j0 + w], in_=ot[:, :w])

            agg_ps = None
            emit_head(0)
            for bi, (b0, bn) in enumerate(batches):
                if bi + 1 < len(batches):
                    emit_head(bi + 1)
                gst, gsTt, zb = heads.pop(bi)

                for w0 in range(0, bn, WIN):
                    wn = min(WIN, bn - w0)
                    W = wn * 128
                    mt_ps = psA.tile([128, 512], f32, tag="psA")
                    for ci in range(wn):
                        k = int(blk_of_chunk[b0 + w0 + ci])
                        nc.tensor.matmul(
                            out=mt_ps[:, ci * 128 : (ci + 1) * 128],
                            lhsT=gsTt[:, (w0 + ci) * 128 : (w0 + ci + 1) * 128],
                            rhs=yvB[:, k * 128 : (k + 1) * 128],
                            start=True,
                            stop=True,
                        )
                    m = wpool.tile([128, WIN * 128], bf16, tag="m")
                    nc.vector.tensor_tensor(
                        out=m[:, :W],
                        in0=mt_ps[:, :W],
                        in1=zb[:, w0 : w0 + wn, :].rearrange("p a b -> p (a b)"),
                        op=ALU.add,
                    )
                    msg = wpool.tile([128, WIN * 128], bf16, tag="msg")
                    nc.scalar.activation(out=msg[:, :W], in_=m[:, :W], func=AF.Relu)

                    for ci in range(wn):
                        g = b0 + w0 + ci
                        k = int(blk_of_chunk[g])
                        first = g == blk_start[k]
                        last = g == blk_start[k + 1] - 1
                        if first:
                            agg_new = psC.tile([128, 128], f32, tag="aggrun")
                            agg_ps = agg_new
                        nc.tensor.matmul(
                            out=agg_ps[:],
                            lhsT=msg[:, ci * 128 : (ci + 1) * 128],
                            rhs=gst[:, (w0 + ci) * 128 : (w0 + ci + 1) * 128],
                            start=first,
                            stop=last,
                        )
                        if last:
                            block_done(k, agg_ps)

    nc.finalize()
    return nc


def _make_in_maps(variables, factors, Wm, bm, Wc, bc, st, core_data):
    vpc, vpad, fpad = st["vpc"], st["vpad"], st["fpad"]

    V = np.asarray(variables, dtype=np.float32)
    F = np.asarray(factors, dtype=np.float32)
    Wm = np.asarray(Wm, dtype=np.float32)
    Wc = np.asarray(Wc, dtype=np.float32)
    bm = np.asarray(bm, dtype=np.float32)
    bc = np.asarray(bc, dtype=np.float32)

    ftp = np.zeros((128, fpad), dtype=BF16)
    ftp[:, : F.shape[0]] = F.T.astype(BF16)

    shared = dict(
        ft=ftp,
        wm_top=Wm[:128, :].astype(BF16),
        wm_bot=Wm[128:, :].astype(BF16),
        wc_top=Wc[:128, :].astype(BF16),
        wc_bot=Wc[128:, :].astype(BF16),
        bm4=np.tile(bm[None, :], (128, 4)).astype(BF16),
        bc_col=bc[:, None].astype(np.float32),
    )

    in_maps = []
    for c in range(N_CORES):
        lo = c * vpc
        vtp = np.zeros((128, vpad), dtype=BF16)
        vtp[:, :vpc] = V[lo : lo + vpc].T.astype(BF16)
        m = dict(shared)
        m["vt"] = vtp
        m["zidx"] = core_data[c]["zidx"]
        m["gs"] = core_data[c]["gs"]
        m["gsT"] = core_data[c]["gsT"]
        in_maps.append(m)
    return in_maps


def kernel(variables, factors, senders, receivers, Wm, bm, Wc, bc, _trace=False):
    from concourse.bass_utils import run_bass_kernel_spmd

    st, core_data = _make_plan(senders, receivers)
    nc = _build_program(st)
    in_maps = _make_in_maps(variables, factors, Wm, bm, Wc, bc, st, core_data)
    res = run_bass_kernel_spmd(
        nc, in_maps, core_ids=list(range(N_CORES)), trace=_trace
    )
    vpc = st["vpc"]
    out = np.concatenate(
        [res.results[c]["out_t"].T[:vpc] for c in range(N_CORES)], axis=0
    )
    if _trace:
        kernel.last_exec_time_ns = res.exec_time_ns
        kernel.last_results = res
    return out.astype(np.float32)
